# revision 25
# baseline (speedup 1.0000x reference)
"""GRU cell on 8 Trainium2 NeuronCores — data-parallel over batch, fp8 matmuls.

Math (per batch row):
    z = sigmoid([x, h] @ W_z + b_z)
    r = sigmoid([x, h] @ W_r + b_r)
    n = tanh(x @ W_n[:D] + (r * h) @ W_n[D:] + b_n)
    h' = (1 - z) * h + z * n = u + z * n   with u = h - z * h

Distribution: batch 8192 is split 1024 rows per core; weights are
replicated. Everything on-device is computed in a transposed layout
[hidden, batch] so both matmul operands have the contraction dim on
SBUF partitions and no on-device transpose is needed:
    out.T[ho, b] = sum_k W[k, ho] * xh.T[k, b]
The host pre-transposes x/h (free) and transposes the result back.

Matmuls run in fp8_e4m3 with DoubleRow perf mode and a full-width
stationary [128, 2, 128]: the PE virtualizes to 256(k) x 128(out)
with 2 fp8 MACs/cell/cycle; one N=512 instruction streams in
512/2.4GHz + ~2.5ns = 216 ns (the measured warm pitch), so the PE
floor is 1536 x 216 = 332 us. The kernel is structured so the MM
stream is gapless from the first real MM to the last:
  - a few dummy warm-up matmuls bridge the PE from its earliest
    start (~8.2 us) to the first data (~9.5 us), so the HAM busy
    window starts counting early; the first real matmuls run during
    the 1.2->2.4 GHz ramp doing useful (cold-clock) work.
  - the first matmul's operands are fine-grained DMA chunks (32 KB
    per-k-pair weight chunks, 256 KB xh k-pairs) at the head of the
    three DMA queues, so the gating bytes are ~380 KB instead of
    ~1.9 MB and the first real matmul starts ~6 us earlier.
  - the R phase interleaves j=0..3 so xh is consumed at ~148 GB/s;
    the sync queue carries xh first (bias/h32 strictly after), and
    weight traffic self-paces to ~49 GB/s per queue via the ~650 ns
    per-descriptor issue rate, so xh supply exceeds consumption.
  - bulk weight prefetch (j4/j5) queues behind gpsimd's fine chunks,
    so it cannot steal HBM bandwidth during the interleave block.
W entries are ~U(-1/64, 1/64), at e4m3's min-normal, so weights are
pre-scaled by 64 on the host and the matmul result is descaled by
1/64 in the activation (out = act(psum/64 + bias)).
The elementwise pipeline runs in bf16. u = h - z*h is precomputed on
the z path (off the critical tail), so after the last n matmul only
act + 2 vector ops + one DMA remain.
"""

import os
import sys
import types

import numpy as np

import concourse.bass as bass
import concourse.tile as tile
from concourse import bacc, mybir
from concourse._compat import with_exitstack
from concourse.bass_interp import get_hw_module
from concourse.bass_utils import run_bass_kernel_spmd

from ml_dtypes import bfloat16 as np_bf16
from ml_dtypes import float8_e4m3 as np_fp8

N_CORES = 8
D = 2048  # input size
H = 2048  # hidden size
BATCH = 8192
BC = BATCH // N_CORES  # batch per core (1024)
K = D + H  # contraction dim (4096)
P = 128  # partitions
KT = K // P  # k-chunks (32)
KK = KT // 2  # double-row k-pairs over [x, h] (16)
DT = D // P  # k-chunks covering the x part (16)
DK = DT // 2  # double-row k-pairs covering the x part (8)
JT = H // P  # hidden-out tiles (16)
NF = 512  # moving free dim per matmul (one PSUM bank of fp32)
NB = BC // NF  # batch blocks per core (2)
WSCALE = 64.0  # host-side weight pre-scale (descaled in activation)
N_WARM = 9  # dummy warm-up matmuls bridging PE start to first data
NI = 4  # leading j-tiles interleaved in the R phase

f32 = mybir.dt.float32
bf16 = mybir.dt.bfloat16
fp8 = mybir.dt.float8e4
DR = mybir.MatmulPerfMode.DoubleRow


def _install_ntff_hook():
    """antenv.axon_hooks isn't injected in this image; shim it so
    run_bass_kernel_spmd(trace=True) can capture NTFF profiles."""
    if "antenv.axon_hooks" in sys.modules:
        return
    try:
        from trn_agent_boot.trn_boot import _ntff_profile_via_ctypes

        hook = _ntff_profile_via_ctypes("/opt/axon/libaxon_pjrt.so")
    except Exception:
        hook = None
    mod = types.ModuleType("antenv.axon_hooks")
    mod.get_axon_ntff_profile_hook = lambda: hook
    mod.set_axon_ntff_profile_hook = lambda h: None
    sys.modules["antenv.axon_hooks"] = mod


@with_exitstack
def _gru_tile_kernel(ctx, tc, xh, h32, wz, wr, wn, ball, out):
    nc = tc.nc
    Sigmoid = mybir.ActivationFunctionType.Sigmoid
    Tanh = mybir.ActivationFunctionType.Tanh

    const_pool = ctx.enter_context(tc.tile_pool(name="const", bufs=1))
    xh_pool = ctx.enter_context(tc.tile_pool(name="xhp", bufs=1))
    h32_pool = ctx.enter_context(tc.tile_pool(name="h32p", bufs=1))
    rh_pool = ctx.enter_context(tc.tile_pool(name="rhp", bufs=1))
    w_pool = ctx.enter_context(tc.tile_pool(name="wp", bufs=12))
    act_pool = ctx.enter_context(tc.tile_pool(name="actp", bufs=2))
    out_pool = ctx.enter_context(tc.tile_pool(name="outp", bufs=3))
    psum_pool = ctx.enter_context(tc.tile_pool(name="psp", bufs=8, space="PSUM"))

    xh_sb = xh_pool.tile([P, KT, BC], fp8, name="xh_sb")
    h32_sb = h32_pool.tile([P, JT, BC], bf16, name="h32_sb")
    # r * h_prev (transposed) in fp8, filled during the r phase.
    rh_sb = rh_pool.tile([P, JT, BC], fp8, name="rh_sb")

    # --- PE warm-up: dummy DoubleRow matmuls on a scratch tile,
    # issued before anything else so the PE is busy from ~8.2 us
    # (memset + cross-engine semaphore latency after the ~6.3 us
    # engine preamble). The HAM clock gate needs ~3.4 us of sustained
    # activity to lift the PE from 1.2 to 2.4 GHz; the dummies start
    # that clock early so the real matmul stream (data-gated at
    # ~10.5 us) reaches full speed sooner. The memset runs on gpsimd
    # ahead of its DMA descriptors (~100 ns); values are irrelevant
    # (results discarded, the psum bank is reset by the first real
    # start=True matmul).
    dummy_sb = const_pool.tile([P, 2, NF], fp8, name="dummy_sb")
    nc.gpsimd.memset(dummy_sb[:], 0.0)
    dummy_ps = psum_pool.tile([P, NF], f32, tag="ps", name="dummy_ps")
    for _ in range(N_WARM):
        nc.tensor.matmul(
            dummy_ps[:],
            dummy_sb[:, :, 0:P],
            dummy_sb[:],
            start=True,
            stop=True,
            perf_mode=DR,
        )

    def load_w_fine2(w_ap, ja, jb, name, eng):
        """Two [128, KT, 128] weight tiles DMA'd as interleaved
        2-k-pair chunks (64 KB each): ja_kk01, jb_kk01, ja_kk23, ...
        The ~650 ns/descriptor issue rate self-paces the queue to
        ~95 GB/s — comfortably ahead of the interleave block's
        ~37 GB/s per-queue weight consumption but far below what an
        unpaced bulk stream would grab from xh — and the first
        matmul gates on 64 KB instead of a whole 512 KB tile."""
        wa = w_pool.tile([P, KT, P], fp8, tag="w", name=name)
        wb = w_pool.tile([P, KT, P], fp8, tag="w", name=name)
        for s in range(8):
            sl = slice(4 * s, 4 * s + 4)
            eng.dma_start(wa[:, sl, :], w_ap[ja, :, sl, :])
            eng.dma_start(wb[:, sl, :], w_ap[jb, :, sl, :])
        return wa, wb

    def load_w_cols(w_ap, j, name, nway=4, eng=None):
        """[128, KT, 128] tile in nway chunks from the gpsimd queue
        (the Sync queue's serial ~600ns-per-descriptor stream stays
        free for xh/h/out)."""
        wt = w_pool.tile([P, KT, P], fp8, tag="w", name=name)
        step = KT // nway
        issuer = eng if eng is not None else nc.gpsimd
        for s in range(nway):
            issuer.dma_start(
                wt[:, s * step : (s + 1) * step, :],
                w_ap[j, :, s * step : (s + 1) * step, :],
            )
        return wt

    # DMA issue order is latency-critical, and each queue is a FIFO
    # that greedily pulls HBM bandwidth — so the critical xh stream
    # gets the sync queue to itself, the per-k-pair weight chunks the
    # first matmuls gate on are naturally paced by the ~650ns-per-
    # descriptor issue rate (~50 GB/s per interleaved pair), and the
    # bulk j4/j5 prefetch queues up BEHIND scalar's fine chunks so it
    # cannot steal bandwidth from xh during the interleave block.
    #   sync:   xh k-pairs 0..15 (fine->bulk), then all of h32
    #   gpsimd: wr_j0 / wr_j1 fine chunks interleaved, later bulk
    #   scalar: biases, wr_j2 / wr_j3 fine chunks, then j4/j5 bulk
    xh_flat = xh_sb[:].rearrange("p t n -> p (t n)")
    cw = 2 * BC  # one k-pair of xh (2 chunks)
    nc.sync.dma_start(xh_flat[:, 0:cw], xh[:, 0:cw])
    nc.sync.dma_start(xh_flat[:, cw : 2 * cw], xh[:, cw : 2 * cw])
    wr_tiles = [None] * 4
    wr_tiles[0], wr_tiles[1] = load_w_fine2(wr, 0, 1, "wr_j", nc.gpsimd)
    wr_tiles[2], wr_tiles[3] = load_w_fine2(wr, 2, 3, "wr_j", nc.scalar)
    # All three biases in one host-packed [128, 48] f32 tensor: one
    # descriptor with 192-byte partition lines. (A `(j p) -> p j`
    # gather of a [H] vector emits 6144 four-byte DMA packets, which
    # wedges the issuing queue's FIFO for ~6 us.)
    bias_t = const_pool.tile([P, 3 * JT], f32, name="bias_all")
    nc.sync.dma_start(bias_t[:], ball[:])
    _boff = {"z": 0, "r": JT, "n": 2 * JT}

    def bcol(g, j):
        return bias_t[:, _boff[g] + j : _boff[g] + j + 1]

    for s in range(1, 8):
        nc.sync.dma_start(
            xh_flat[:, s * 2 * cw : (s + 1) * 2 * cw], xh[:, s * 2 * cw : (s + 1) * 2 * cw]
        )
    h32_flat = h32_sb[:].rearrange("p t n -> p (t n)")
    for s in range(8):
        w = JT * BC // 8
        nc.sync.dma_start(h32_flat[:, s * w : (s + 1) * w], h32[:, s * w : (s + 1) * w])

    wr_tiles.append(load_w_cols(wr, 4, "wr_j"))
    wr_tiles.append(load_w_cols(wr, 5, "wr_j"))

    def accumulate(ps, w_tile, rhs_of_kk):
        """DoubleRow with full-width stationary [128, 2, 128]: the PE
        virtualizes to 256(k) x 128(out), 2 fp8 MACs/cell/cycle.
        16 k-pairs x 2 b_i accumulate into ps[b_i] [128, NF]."""
        for kk in range(KK):
            lhsT = w_tile[:, 2 * kk : 2 * kk + 2, :]
            for b_i in range(NB):
                nc.tensor.matmul(
                    ps[b_i][:],
                    lhsT,
                    rhs_of_kk(kk, b_i),
                    start=(kk == 0),
                    stop=(kk == KK - 1),
                    perf_mode=DR,
                )

    def xh_rhs(kk, b_i):
        return xh_sb[:, 2 * kk : 2 * kk + 2, b_i * NF : (b_i + 1) * NF]

    def n_rhs(kk, b_i):
        if kk < DK:
            return xh_rhs(kk, b_i)
        tt = kk - DK
        return rh_sb[:, 2 * tt : 2 * tt + 2, b_i * NF : (b_i + 1) * NF]

    def new_ps(name):
        return [psum_pool.tile([P, NF], f32, tag="ps", name=name) for _ in range(NB)]

    def act_gate(dst, ps, func, bias_col):
        """dst [128, BC] <- act(ps/WSCALE + bias)."""
        for b_i in range(NB):
            nc.scalar.activation(
                dst[:, b_i * NF : (b_i + 1) * NF],
                ps[b_i][:],
                func,
                bias=bias_col,
                scale=1.0 / WSCALE,
            )

    # ---- phase R: r gate, then rh = r * h_prev ----
    # j=0..3 interleave their k-loops: one j-tile alone has only
    # ~450 ns of ready matmul work per arriving xh k-pair (vs the
    # ~1.4 us k-pair DMA cadence); four together consume xh at
    # ~148 GB/s, just under what the sync queue sustains, so the PE
    # never starves while the 4 MB xh stream lands.
    ps01 = [new_ps("ps_r01") for _ in range(NI)]
    for kk in range(KK):
        for jj in range(NI):
            lhsT = wr_tiles[jj][:, 2 * kk : 2 * kk + 2, :]
            for b_i in range(NB):
                nc.tensor.matmul(
                    ps01[jj][b_i][:],
                    lhsT,
                    xh_rhs(kk, b_i),
                    start=(kk == 0),
                    stop=(kk == KK - 1),
                    perf_mode=DR,
                )
    for jj in range(NI):
        r_j = act_pool.tile([P, BC], bf16, tag="r", name="r_j")
        act_gate(r_j, ps01[jj], Sigmoid, bcol("r", jj))
        nc.vector.tensor_mul(rh_sb[:, jj, :], r_j[:], h32_sb[:, jj, :])

    wz_tiles, wn_tiles = [], []
    for j in range(NI, JT):
        wr_j = wr_tiles[j]
        if j + 2 < JT:
            wr_tiles.append(load_w_cols(wr, j + 2, "wr_j"))
        if j >= JT - 4:
            idx = j - (JT - 4)
            wz_tiles.append(load_w_cols(wz, idx, "wz_j"))
            wn_tiles.append(load_w_cols(wn, idx, "wn_j"))
        ps = new_ps("ps_r")
        accumulate(ps, wr_j, xh_rhs)
        r_j = act_pool.tile([P, BC], bf16, tag="r", name="r_j")
        act_gate(r_j, ps, Sigmoid, bcol("r", j))
        nc.vector.tensor_mul(rh_sb[:, j, :], r_j[:], h32_sb[:, j, :])

    # ---- phase NZ: z and n gates + combine ----
    for j in range(JT):
        wz_j, wn_j = wz_tiles[j], wn_tiles[j]
        if j + 4 < JT:
            wz_tiles.append(load_w_cols(wz, j + 4, "wz_j"))
            wn_tiles.append(load_w_cols(wn, j + 4, "wn_j"))
        # z fully accumulates + activates before n's psum tiles are
        # claimed, so 8 banks still give j-to-j+1 double buffering.
        z_j = act_pool.tile([P, BC], bf16, tag="z", name="z_j")
        n_j = act_pool.tile([P, BC], bf16, tag="n", name="n_j")
        psz = new_ps("ps_z")
        accumulate(psz, wz_j, xh_rhs)
        act_gate(z_j, psz, Sigmoid, bcol("z", j))
        # u = h - z*h = (1-z)*h, precomputed here (overlaps n's
        # matmuls) so the n-side combine is only 2 vector ops.
        zh_j = act_pool.tile([P, BC], bf16, tag="zh", name="zh_j")
        nc.vector.tensor_mul(zh_j[:], z_j[:], h32_sb[:, j, :])
        u_j = act_pool.tile([P, BC], bf16, tag="u", name="u_j")
        nc.vector.tensor_sub(u_j[:], h32_sb[:, j, :], zh_j[:])
        psn = new_ps("ps_n")
        if j == JT - 1:
            # Last iteration: run the two batch halves' n-accumulations
            # back-to-back instead of interleaved, so half 0's
            # activation + combine + output DMA overlap half 1's
            # matmuls and only half the work trails the final matmul.
            for b_i in range(NB):
                for kk in range(KK):
                    nc.tensor.matmul(
                        psn[b_i][:],
                        wn_j[:, 2 * kk : 2 * kk + 2, :],
                        n_rhs(kk, b_i),
                        start=(kk == 0),
                        stop=(kk == KK - 1),
                        perf_mode=DR,
                    )
                if b_i == NB - 1:
                    # The kernel's tail: split the final activation and
                    # combine into two 256-column pieces so vector work
                    # starts as soon as the first half-activation is
                    # done; each piece's DMA goes out on its own queue
                    # (scalar then sync) so the two ~1.3 us descriptor-
                    # to-completion latencies overlap.
                    PIECE = NF // 2
                    for s in range(2):
                        psl = slice(b_i * NF + s * PIECE, b_i * NF + (s + 1) * PIECE)
                        nc.scalar.activation(
                            n_j[:, psl],
                            psn[b_i][:, s * PIECE : (s + 1) * PIECE],
                            Tanh,
                            bias=bcol("n", j),
                            scale=1.0 / WSCALE,
                        )
                        zn_p = act_pool.tile([P, PIECE], bf16, tag="znp", name="zn_p")
                        nc.vector.tensor_mul(zn_p[:], z_j[:, psl], n_j[:, psl])
                        o_p = out_pool.tile([P, PIECE], bf16, name="o_p")
                        nc.vector.tensor_add(o_p[:], zn_p[:], u_j[:, psl])
                        issuer = nc.scalar if s == 0 else nc.sync
                        issuer.dma_start(out[j * P : (j + 1) * P, psl], o_p[:])
                    continue
                nc.scalar.activation(
                    n_j[:, b_i * NF : (b_i + 1) * NF],
                    psn[b_i][:],
                    Tanh,
                    bias=bcol("n", j),
                    scale=1.0 / WSCALE,
                )
                sl = slice(b_i * NF, (b_i + 1) * NF)
                zn_j = act_pool.tile([P, NF], bf16, tag="zn", name="zn_j")
                nc.vector.tensor_mul(zn_j[:], z_j[:, sl], n_j[:, sl])
                o_j = out_pool.tile([P, NF], bf16, name="o_j")
                nc.vector.tensor_add(o_j[:], zn_j[:], u_j[:, sl])
                nc.sync.dma_start(out[j * P : (j + 1) * P, sl], o_j[:])
            continue
        accumulate(psn, wn_j, n_rhs)
        act_gate(n_j, psn, Tanh, bcol("n", j))

        # h' = u + z * n, per batch half for finer overlap of the
        # combine + output DMA with the next j's matmuls.
        for b_i in range(NB):
            sl = slice(b_i * NF, (b_i + 1) * NF)
            zn_j = act_pool.tile([P, NF], bf16, tag="zn", name="zn_j")
            nc.vector.tensor_mul(zn_j[:], z_j[:, sl], n_j[:, sl])
            o_j = out_pool.tile([P, NF], bf16, name="o_j")
            nc.vector.tensor_add(o_j[:], zn_j[:], u_j[:, sl])
            nc.sync.dma_start(out[j * P : (j + 1) * P, sl], o_j[:])


_CACHED = None


def _build():
    global _CACHED
    if _CACHED is not None:
        return _CACHED
    nc = bacc.Bacc(
        "TRN2", target_bir_lowering=False, debug=False, enable_asserts=False
    )
    # xh/h32 arrive pre-packed partition-major ([P, chunks*BC]) so the
    # per-partition DMA lines are long and contiguous.
    xh = nc.dram_tensor("xh", [P, KT * BC], fp8, kind="ExternalInput").ap()
    h32 = nc.dram_tensor("h32", [P, JT * BC], bf16, kind="ExternalInput").ap()
    # Weights pre-arranged on host: [JT, P, KT, P] where
    # w[j, p, t, m] = W[t*128+p, j*128+m] * 64, so the per-j DMA is a
    # fully contiguous [128, 4096] block (4 KiB per partition line).
    wz = nc.dram_tensor("wz", [JT, P, KT, P], fp8, kind="ExternalInput").ap()
    wr = nc.dram_tensor("wr", [JT, P, KT, P], fp8, kind="ExternalInput").ap()
    wn = nc.dram_tensor("wn", [JT, P, KT, P], fp8, kind="ExternalInput").ap()
    # All three biases host-packed as [P, 3*JT] f32 with
    # ball[p, g*JT + j] = b_g[j*128 + p]: one contiguous DMA with
    # 192-byte partition lines instead of 6144 four-byte packets.
    ball = nc.dram_tensor("ball", [P, 3 * JT], f32, kind="ExternalInput").ap()
    out = nc.dram_tensor("out", [H, BC], bf16, kind="ExternalOutput").ap()

    with tile.TileContext(nc) as tc:
        _gru_tile_kernel(tc, xh, h32, wz, wr, wn, ball, out)
    nc.compile()
    nc.m = get_hw_module(nc.m)
    _CACHED = nc
    return nc


def _pack_weight(W):
    """[K, H] f32 -> [JT, P, KT, P] fp8 with w[j,p,t,m] = W[t*128+p, j*128+m]*64."""
    w8 = (np.asarray(W, np.float32) * WSCALE).astype(np_fp8)
    w8 = w8.reshape(KT, P, JT, P).transpose(2, 1, 0, 3)
    return np.ascontiguousarray(w8)


def _make_in_maps(x, h_prev, W_z, b_z, W_r, b_r, W_n, b_n):
    wz8 = _pack_weight(W_z)
    wr8 = _pack_weight(W_r)
    wn8 = _pack_weight(W_n)
    # ball[p, g*JT + j] = b_g[j*128 + p]
    ball = np.stack(
        [np.asarray(b, np.float32).reshape(JT, P).T for b in (b_z, b_r, b_n)],
        axis=1,
    ).reshape(P, 3 * JT)
    ball = np.ascontiguousarray(ball)
    in_maps = []
    for i in range(N_CORES):
        sl = slice(i * BC, (i + 1) * BC)
        xt = np.asarray(x[sl], np.float32).T
        ht = np.asarray(h_prev[sl], np.float32).T
        xh_i = np.concatenate([xt, ht], axis=0).astype(np_fp8)
        # pack [K, BC] -> [P, KT*BC] and [H, BC] -> [P, JT*BC]
        xh_p = xh_i.reshape(KT, P, BC).transpose(1, 0, 2).reshape(P, KT * BC)
        h16 = ht.astype(np_bf16)
        h32_p = h16.reshape(JT, P, BC).transpose(1, 0, 2).reshape(P, JT * BC)
        in_maps.append(
            {
                "xh": np.ascontiguousarray(xh_p),
                "h32": np.ascontiguousarray(h32_p),
                "wz": wz8,
                "wr": wr8,
                "wn": wn8,
                "ball": ball,
            }
        )
    return in_maps


LAST_RESULT = None


def kernel(x, h_prev, W_z, b_z, W_r, b_r, W_n, b_n):
    global LAST_RESULT
    trace = bool(os.environ.get("GRU_TRACE"))
    if trace:
        _install_ntff_hook()
    nc = _build()
    in_maps = _make_in_maps(x, h_prev, W_z, b_z, W_r, b_r, W_n, b_n)
    res = run_bass_kernel_spmd(
        nc, in_maps, core_ids=list(range(N_CORES)), trace=trace
    )
    LAST_RESULT = res
    outs = [res.results[i]["out"].T for i in range(N_CORES)]
    return np.ascontiguousarray(np.concatenate(outs, axis=0).astype(np.float32))
